# revision 42
# baseline (speedup 1.0000x reference)
"""Trainium2 Bass kernel for nn_ClassificationLoss (NMS-detection CE loss).

Data-parallel across 8 NeuronCores (2 images each) with a spatially
binned IoU grid:

Host prep (per image): preds are sorted into 126 spatial cells (7 x-sorted
columns x 18 y-sorted rows, 200 preds each = one SBUF partition per cell).
For each cell only GT boxes that could reach IoU>=0.4 with some pred in the
cell (exact interval/area necessity test with 3% slack) are kept, ranked,
and truncated/padded to MPAD=8 slots.  The host ships fp16 feature rows:
per-pred (x2, -x1, y2, -y1, area/3.5), per-pred scores, the per-cell GT
table (x2, -x1, y2, -y1, area/3.5), and S[n,j] = score of pred n at the
class of candidate j (+16 offset) so the kernel never needs a per-lane
gather.

Device math (validated vs reference, rel err ~3e-5):
  crosses_j = [ relu(min(px2,gx2)+min(-px1,-gx1)) * (min(py2,gy2)+min(-py1,-gy1))
                - pa/3.5 >= ga/3.5 ]            (iou>=0.4 without any division)
  smax  = max_j crosses_j * (MPAD-j)            (slot selection, fp16-exact)
  sl+16 = max_j [blc==smax] * (S_nj+16)         (score at selected slot)
  ce    = (ln(sum_c exp(s_c)) + 16) - (sl+16);  loss = masked mean (host finish)

Engines: DVE runs the fp16 grid (2x packed mode) + CE halving trees,
GpSimd(Pool) takes the min/is_ge/max grid ops, Activation does Exp/Ln.
"""

import numpy as np
import ml_dtypes

import concourse.bass as bass
import concourse.bacc as bacc
import concourse.tile as tile
import concourse.mybir as mybir
from concourse.bass_utils import run_bass_kernel_spmd

B, N, C, M = 16, 25200, 80, 64
NCORES = 8
IMGS_PER_CORE = B // NCORES          # 2
CX, CY = 7, 18
P = CX * CY                          # 126 partitions = cells
ROWS = N // P                        # 200 preds per cell
NCHUNK = 2
K = ROWS // NCHUNK                   # 100 preds per chunk
MPAD = 4                             # GT candidate slots per cell
THR = float(np.float64(2.0) / np.float64(7.0))
DGA = 60000.0                        # dummy slot ga'   (never crossed)

F32 = mybir.dt.float32
F16 = mybir.dt.float16
F8 = mybir.dt.float8e4
I32 = mybir.dt.int32
Alu = mybir.AluOpType
Act = mybir.ActivationFunctionType
AX = mybir.AxisListType

_CACHE = {}


def _bc(ap_like, extra_offset, dims):
    """Raw AP with explicit [step, count] dims (0-step = broadcast)."""
    return bass.AP(tensor=ap_like.tensor, offset=ap_like.offset + extra_offset, ap=dims)


def _build():
    nc = bacc.Bacc("TRN2")
    c_in = nc.dram_tensor("c", [IMGS_PER_CORE, P, 4, ROWS], F16, kind="ExternalInput")
    s_in = nc.dram_tensor("s", [IMGS_PER_CORE, P, ROWS, C], F8, kind="ExternalInput")
    sg_in = nc.dram_tensor("sg", [IMGS_PER_CORE, P, MPAD, ROWS], F16, kind="ExternalInput")
    pg_in = nc.dram_tensor("pg", [IMGS_PER_CORE, P, MPAD, ROWS], F16, kind="ExternalInput")
    g_in = nc.dram_tensor("g", [IMGS_PER_CORE, P, 4, MPAD], F16, kind="ExternalInput")
    o_se = nc.dram_tensor("ose", [IMGS_PER_CORE, P, ROWS], F32, kind="ExternalOutput")
    o_sl = nc.dram_tensor("osl", [IMGS_PER_CORE, P, ROWS], F16, kind="ExternalOutput")
    o_sm = nc.dram_tensor("osm", [IMGS_PER_CORE, P, ROWS], F16, kind="ExternalOutput")

    with tile.TileContext(nc) as tc:
        with (
            tc.tile_pool(name="chunkp", bufs=3) as chunkp,
            tc.tile_pool(name="gridp", bufs=3) as gridp,
            tc.tile_pool(name="singles", bufs=1) as singles,
            tc.tile_pool(name="imgp", bufs=1) as imgp,
        ):
            # slot code MPAD-j, replicated over K (compile-time constant)
            code_i = singles.tile([P, MPAD, K], I32)
            nc.gpsimd.iota(code_i, pattern=[[-1, MPAD], [0, K]], base=MPAD,
                           channel_multiplier=0)
            codeT = singles.tile([P, MPAD, K], F16)
            nc.vector.tensor_copy(codeT, code_i)

            smax_b, sl_b, se_b = [], [], []
            for b in range(IMGS_PER_CORE):
                gt = imgp.tile([P, 4, MPAD], F16, tag=f"gt{b}")
                nc.sync.dma_start(out=gt, in_=g_in[b])
                ct = imgp.tile([P, 4, ROWS], F16, tag=f"ct{b}")
                nc.sync.dma_start(out=ct, in_=c_in[b])
                sgt = imgp.tile([P, MPAD, ROWS], F16, tag=f"sgt{b}")
                nc.sync.dma_start(out=sgt, in_=sg_in[b])
                pgt = imgp.tile([P, MPAD, ROWS], F16, tag=f"pgt{b}")
                nc.sync.dma_start(out=pgt, in_=pg_in[b])

                # materialize GT coord rows into one stacked [P, 4, MPAD, K]
                # grid (K-replicated) for the fused min
                gt4T = imgp.tile([P, 4, MPAD, K], F16, tag=f"gt4{b}")
                src = gt[:, :, :]
                nc.gpsimd.tensor_copy(
                    gt4T, _bc(src, 0, [src.ap[0], [MPAD, 4], [1, MPAD], [0, K]])
                )

                smax_i = imgp.tile([P, ROWS], F16, tag=f"smax{b}")
                sl_i = imgp.tile([P, ROWS], F16, tag=f"sl{b}")
                se_i = imgp.tile([P, ROWS], F32, tag=f"se{b}")
                smax_b.append(smax_i); sl_b.append(sl_i); se_b.append(se_i)

                for k in range(NCHUNK):
                    c0 = k * K

                    # ---- IoU threshold grid: fused 4-coordinate min + paired add
                    mm = gridp.tile([P, 4, MPAD, K], F16, tag="mm")
                    ca = ct[:, :, :]
                    pred4B = _bc(ca, c0, [ca.ap[0], [ROWS, 4], [0, MPAD], [1, K]])
                    nc.vector.tensor_tensor(mm, pred4B, gt4T[:, :, :, :], op=Alu.min)
                    wh = gridp.tile([P, 2, MPAD, K], F16, tag="wh")
                    ma = mm[:, :, :, :]
                    ev = _bc(ma, 0, [ma.ap[0], [2 * MPAD * K, 2], [K, MPAD], [1, K]])
                    od = _bc(ma, MPAD * K, [ma.ap[0], [2 * MPAD * K, 2], [K, MPAD], [1, K]])
                    nc.vector.tensor_tensor(wh, ev, od, op=Alu.add)
                    wr = gridp.tile([P, MPAD, K], F16, tag="wr")
                    nc.vector.tensor_scalar(wr, wh[:, 0, :, :], 0.0, None, op0=Alu.max)
                    ii = gridp.tile([P, MPAD, K], F16, tag="ii")
                    nc.vector.tensor_tensor(ii, wr, wh[:, 1, :, :], op=Alu.mult)
                    bx = gridp.tile([P, MPAD, K], F16, tag="bx")
                    pga = pgt[:, :, :]
                    pgB = _bc(pga, c0, [pga.ap[0], [ROWS, MPAD], [1, K]])
                    nc.vector.tensor_tensor(bx, ii, pgB, op=Alu.is_ge)
                    blc = gridp.tile([P, MPAD, K], F16, tag="blc")
                    nc.vector.tensor_tensor(blc, bx, codeT[:, :, :], op=Alu.mult)

                    # ---- slot-code max tree 4 -> 2 -> 1
                    t2 = gridp.tile([P, 2, K], F16, tag="t2")
                    nc.vector.tensor_tensor(t2, blc[:, 0:2, :], blc[:, 2:4, :], op=Alu.max)
                    nc.vector.tensor_tensor(
                        smax_i[:, c0:c0 + K], t2[:, 0, :], t2[:, 1, :], op=Alu.max
                    )

                    # ---- select S at winning slot: max_j [blc==smax]*(S+16)
                    sm = smax_i[:, :]
                    smB = _bc(sm, c0, [sm.ap[0], [0, MPAD], [1, K]])
                    eq = gridp.tile([P, MPAD, K], F16, tag="eq")
                    nc.vector.tensor_tensor(eq, blc, smB, op=Alu.is_equal)
                    slw = gridp.tile([P, MPAD, K], F16, tag="slw")
                    sga = sgt[:, :, :]
                    sgB = _bc(sga, c0, [sga.ap[0], [ROWS, MPAD], [1, K]])
                    nc.vector.tensor_tensor(slw, eq, sgB, op=Alu.mult)
                    s2 = gridp.tile([P, 2, K], F16, tag="s2")
                    nc.vector.tensor_tensor(s2, slw[:, 0:2, :], slw[:, 2:4, :], op=Alu.max)
                    nc.vector.tensor_tensor(
                        sl_i[:, c0:c0 + K], s2[:, 0, :], s2[:, 1, :], op=Alu.max
                    )

                    # ---- CE: exp + halving-tree sum over 80 classes,
                    # in half-chunks so DMA/Act/DVE pipeline finely
                    KH = K // 2
                    for hk in range(2):
                        h0 = c0 + hk * KH
                        sck = chunkp.tile([P, KH, C], F8, tag="sck")
                        nc.sync.dma_start(out=sck, in_=s_in[b, :, h0:h0 + KH, :])
                        esc = chunkp.tile([P, KH, C], F16, tag="esc")
                        nc.scalar.activation(esc, sck, Act.Exp)
                        e40 = chunkp.tile([P, KH, 40], F16, tag="e40")
                        nc.vector.tensor_tensor(e40, esc[:, :, 0:40], esc[:, :, 40:80], op=Alu.add)
                        e20 = chunkp.tile([P, KH, 20], F16, tag="e20")
                        nc.vector.tensor_tensor(e20, e40[:, :, 0:20], e40[:, :, 20:40], op=Alu.add)
                        e10 = chunkp.tile([P, KH, 10], F16, tag="e10")
                        nc.vector.tensor_tensor(e10, e20[:, :, 0:10], e20[:, :, 10:20], op=Alu.add)
                        e5 = chunkp.tile([P, KH, 5], F16, tag="e5")
                        nc.vector.tensor_tensor(e5, e10[:, :, 0:5], e10[:, :, 5:10], op=Alu.add)
                        nc.vector.reduce_sum(se_i[:, h0:h0 + KH], e5, axis=AX.X)

                # ship per-pred (sl+16, smax, se) rows when the image finishes
                # (on the idle Pool DMA queue so input DMAs on the SP queue are
                # never blocked); host does ln + masked mean
                nc.gpsimd.dma_start(out=o_sl[b], in_=sl_i)
                nc.gpsimd.dma_start(out=o_sm[b], in_=smax_i)
                nc.gpsimd.dma_start(out=o_se[b], in_=se_i)

    nc.compile()
    return nc


def _host_prep(preds, gtruths):
    """Spatial binning + fp16 feature building for all B images."""
    T = THR
    c_all = np.zeros((B, P, 4, ROWS), dtype=np.float16)
    s_all = np.zeros((B, P, ROWS, C), dtype=ml_dtypes.float8_e4m3)
    sg_all = np.zeros((B, P, MPAD, ROWS), dtype=np.float16)
    pg_all = np.zeros((B, P, MPAD, ROWS), dtype=np.float16)
    g_all = np.zeros((B, P, 4, MPAD), dtype=np.float16)
    for b in range(B):
        pb = preds[b, :, :4].astype(np.float64)
        sc = preds[b, :, 5:]
        g = gtruths[b, :, :4].astype(np.float64)
        gcls = gtruths[b, :, 4].astype(np.int64)
        pa = (pb[:, 2] - pb[:, 0]) * (pb[:, 3] - pb[:, 1])
        ga = (g[:, 2] - g[:, 0]) * (g[:, 3] - g[:, 1])
        cxc = (pb[:, 0] + pb[:, 2]) * 0.5
        ordx = np.argsort(cxc, kind="stable")
        cell_id = 0
        for i in range(CX):
            col = ordx[i * (N // CX):(i + 1) * (N // CX)]
            cyc = (pb[col, 1] + pb[col, 3]) * 0.5
            ordy = col[np.argsort(cyc, kind="stable")]
            for j in range(CY):
                cell = ordy[j * ROWS:(j + 1) * ROWS]
                x1, y1 = pb[cell, 0].min(), pb[cell, 1].min()
                x2, y2 = pb[cell, 2].max(), pb[cell, 3].max()
                wx = np.minimum(x2, g[:, 2]) - np.maximum(x1, g[:, 0])
                wy = np.minimum(y2, g[:, 3]) - np.maximum(y1, g[:, 1])
                ovl = np.clip(wx, 0, None) * np.clip(wy, 0, None)
                pamin = pa[cell].min()
                cand = (
                    (wx > 0) & (wy > 0)
                    & (ovl >= 0.97 * T * (pamin + ga))
                    & (ga * (1 - 0.97 * T) >= 0.97 * T * pamin)
                )
                idx = np.where(cand)[0]
                rank = ovl[idx] / (pamin + ga[idx])
                keep = idx[np.argsort(-rank)][:MPAD]
                nk = len(keep)
                c_all[b, cell_id, 0, :] = pb[cell, 2]
                c_all[b, cell_id, 1, :] = -pb[cell, 0]
                c_all[b, cell_id, 2, :] = pb[cell, 3]
                c_all[b, cell_id, 3, :] = -pb[cell, 1]
                s_all[b, cell_id, :, :] = sc[cell]
                gap_full = np.full(MPAD, DGA)
                gtab = g_all[b, cell_id]
                if nk:
                    gtab[0, :nk] = g[keep, 2]
                    gtab[1, :nk] = -g[keep, 0]
                    gtab[2, :nk] = g[keep, 3]
                    gtab[3, :nk] = -g[keep, 1]
                    gap_full[:nk] = ga[keep] / 3.5
                    sg_all[b, cell_id, :nk, :] = (sc[np.ix_(cell, gcls[keep])] + 16.0).T
                pg_all[b, cell_id, :, :] = gap_full[:, None] + (pa[cell] / 3.5)[None, :]
                cell_id += 1
    return c_all, s_all, sg_all, pg_all, g_all


def kernel(preds: np.ndarray, gtruths: np.ndarray) -> np.ndarray:
    if "nc" not in _CACHE:
        _CACHE["nc"] = _build()
    nc = _CACHE["nc"]

    preds = np.ascontiguousarray(preds, dtype=np.float32)
    gtruths = np.ascontiguousarray(gtruths, dtype=np.float32)
    c_all, s_all, sg_all, pg_all, g_all = _host_prep(preds, gtruths)

    in_maps = [
        {
            "c": c_all[c * IMGS_PER_CORE:(c + 1) * IMGS_PER_CORE],
            "s": s_all[c * IMGS_PER_CORE:(c + 1) * IMGS_PER_CORE],
            "sg": sg_all[c * IMGS_PER_CORE:(c + 1) * IMGS_PER_CORE],
            "pg": pg_all[c * IMGS_PER_CORE:(c + 1) * IMGS_PER_CORE],
            "g": g_all[c * IMGS_PER_CORE:(c + 1) * IMGS_PER_CORE],
        }
        for c in range(NCORES)
    ]
    res = run_bass_kernel_spmd(nc, in_maps, core_ids=list(range(NCORES)))
    _CACHE["last_result"] = res

    per_img = []
    for c in range(NCORES):
        r = res.results[c]
        for b in range(IMGS_PER_CORE):
            se = r["ose"][b].astype(np.float64)          # [P, ROWS]
            sl16 = r["osl"][b].astype(np.float64)        # sl + 16
            smax = r["osm"][b].astype(np.float64)
            valid = smax >= 0.5
            ce = (np.log(se) + 16.0) - sl16
            cnt = float(valid.sum())
            per_img.append(float((ce * valid).sum()) / max(cnt, 1.0))
    return np.asarray(np.mean(per_img), dtype=np.float32)


# revision 44
# speedup vs baseline: 1.0622x; 1.0622x over previous
"""Trainium2 Bass kernel for nn_ClassificationLoss (NMS-detection CE loss).

Data-parallel across 8 NeuronCores (2 images each) with a spatially
binned IoU grid:

Host prep (per image): preds are sorted into 126 spatial cells (7 x-sorted
columns x 18 y-sorted rows, 200 preds each = one SBUF partition per cell).
For each cell only GT boxes that could reach IoU>=0.4 with some pred in the
cell (exact interval/area necessity test with 3% slack) are kept, ranked,
and truncated/padded to MPAD=8 slots.  The host ships fp16 feature rows:
per-pred (x2, -x1, y2, -y1, area/3.5), per-pred scores, the per-cell GT
table (x2, -x1, y2, -y1, area/3.5), and S[n,j] = score of pred n at the
class of candidate j (+16 offset) so the kernel never needs a per-lane
gather.

Device math (validated vs reference, rel err ~3e-5):
  crosses_j = [ relu(min(px2,gx2)+min(-px1,-gx1)) * (min(py2,gy2)+min(-py1,-gy1))
                - pa/3.5 >= ga/3.5 ]            (iou>=0.4 without any division)
  smax  = max_j crosses_j * (MPAD-j)            (slot selection, fp16-exact)
  sl+16 = max_j [blc==smax] * (S_nj+16)         (score at selected slot)
  ce    = (ln(sum_c exp(s_c)) + 16) - (sl+16);  loss = masked mean (host finish)

Engines: DVE runs the fp16 grid (2x packed mode) + CE halving trees,
GpSimd(Pool) takes the min/is_ge/max grid ops, Activation does Exp/Ln.
"""

import numpy as np
import ml_dtypes

import concourse.bass as bass
import concourse.bacc as bacc
import concourse.tile as tile
import concourse.mybir as mybir
from concourse.bass_utils import run_bass_kernel_spmd

B, N, C, M = 16, 25200, 80, 64
NCORES = 8
IMGS_PER_CORE = B // NCORES          # 2
CX, CY = 7, 18
P = CX * CY                          # 126 partitions = cells
ROWS = N // P                        # 200 preds per cell
NCHUNK = 2
K = ROWS // NCHUNK                   # 100 preds per chunk
MPAD = 4                             # GT candidate slots per cell
THR = float(np.float64(2.0) / np.float64(7.0))
DGA = 60000.0                        # dummy slot ga'   (never crossed)

F32 = mybir.dt.float32
F16 = mybir.dt.float16
F8 = mybir.dt.float8e4
I32 = mybir.dt.int32
Alu = mybir.AluOpType
Act = mybir.ActivationFunctionType
AX = mybir.AxisListType

_CACHE = {}


def _bc(ap_like, extra_offset, dims):
    """Raw AP with explicit [step, count] dims (0-step = broadcast)."""
    return bass.AP(tensor=ap_like.tensor, offset=ap_like.offset + extra_offset, ap=dims)


def _build():
    nc = bacc.Bacc("TRN2")
    c_in = nc.dram_tensor("c", [IMGS_PER_CORE, P, 4, ROWS], F16, kind="ExternalInput")
    s_in = nc.dram_tensor("s", [IMGS_PER_CORE, P, ROWS, C], F8, kind="ExternalInput")
    sg_in = nc.dram_tensor("sg", [IMGS_PER_CORE, P, MPAD, ROWS], F16, kind="ExternalInput")
    pg_in = nc.dram_tensor("pg", [IMGS_PER_CORE, P, MPAD, ROWS], F16, kind="ExternalInput")
    g_in = nc.dram_tensor("g", [IMGS_PER_CORE, P, 4, MPAD], F16, kind="ExternalInput")
    o_se = nc.dram_tensor("ose", [IMGS_PER_CORE, P, ROWS], F32, kind="ExternalOutput")
    o_sl = nc.dram_tensor("osl", [IMGS_PER_CORE, P, ROWS], F16, kind="ExternalOutput")
    o_sm = nc.dram_tensor("osm", [IMGS_PER_CORE, P, ROWS], F16, kind="ExternalOutput")

    with tile.TileContext(nc) as tc:
        with (
            tc.tile_pool(name="chunkp", bufs=3) as chunkp,
            tc.tile_pool(name="gridp", bufs=3) as gridp,
            tc.tile_pool(name="singles", bufs=1) as singles,
            tc.tile_pool(name="imgp", bufs=1) as imgp,
        ):
            # slot code MPAD-j, replicated over K (compile-time constant)
            code_i = singles.tile([P, MPAD, K], I32)
            nc.gpsimd.iota(code_i, pattern=[[-1, MPAD], [0, K]], base=MPAD,
                           channel_multiplier=0)
            codeT = singles.tile([P, MPAD, K], F16)
            nc.vector.tensor_copy(codeT, code_i)

            smax_b, sl_b, se_b = [], [], []
            for b in range(IMGS_PER_CORE):
                gt = imgp.tile([P, 4, MPAD], F16, tag=f"gt{b}")
                nc.sync.dma_start(out=gt, in_=g_in[b])
                ct = imgp.tile([P, 4, ROWS], F16, tag=f"ct{b}")
                nc.sync.dma_start(out=ct, in_=c_in[b])
                sgt = imgp.tile([P, MPAD, ROWS], F16, tag=f"sgt{b}")
                nc.sync.dma_start(out=sgt, in_=sg_in[b])
                pgt = imgp.tile([P, MPAD, ROWS], F16, tag=f"pgt{b}")
                nc.sync.dma_start(out=pgt, in_=pg_in[b])

                # materialize GT coord rows into one stacked [P, 4, MPAD, K]
                # grid (K-replicated) for the fused min
                gt4T = imgp.tile([P, 4, MPAD, K], F16, tag=f"gt4{b}")
                src = gt[:, :, :]
                mat_eng = nc.vector if b == 0 else nc.gpsimd  # img0 on DVE: fills idle head
                mat_eng.tensor_copy(
                    gt4T, _bc(src, 0, [src.ap[0], [MPAD, 4], [1, MPAD], [0, K]])
                )

                smax_i = imgp.tile([P, ROWS], F16, tag=f"smax{b}")
                sl_i = imgp.tile([P, ROWS], F16, tag=f"sl{b}")
                se_i = imgp.tile([P, ROWS], F32, tag=f"se{b}")
                smax_b.append(smax_i); sl_b.append(sl_i); se_b.append(se_i)

                for k in range(NCHUNK):
                    c0 = k * K

                    # ---- IoU threshold grid: fused 4-coordinate min + paired add
                    mm = gridp.tile([P, 4, MPAD, K], F16, tag="mm")
                    ca = ct[:, :, :]
                    pred4B = _bc(ca, c0, [ca.ap[0], [ROWS, 4], [0, MPAD], [1, K]])
                    nc.vector.tensor_tensor(mm, pred4B, gt4T[:, :, :, :], op=Alu.min)
                    wh = gridp.tile([P, 2, MPAD, K], F16, tag="wh")
                    ma = mm[:, :, :, :]
                    ev = _bc(ma, 0, [ma.ap[0], [2 * MPAD * K, 2], [K, MPAD], [1, K]])
                    od = _bc(ma, MPAD * K, [ma.ap[0], [2 * MPAD * K, 2], [K, MPAD], [1, K]])
                    nc.vector.tensor_tensor(wh, ev, od, op=Alu.add)
                    wr = gridp.tile([P, MPAD, K], F16, tag="wr")
                    nc.vector.tensor_scalar(wr, wh[:, 0, :, :], 0.0, None, op0=Alu.max)
                    ii = gridp.tile([P, MPAD, K], F16, tag="ii")
                    nc.vector.tensor_tensor(ii, wr, wh[:, 1, :, :], op=Alu.mult)
                    bx = gridp.tile([P, MPAD, K], F16, tag="bx")
                    pga = pgt[:, :, :]
                    pgB = _bc(pga, c0, [pga.ap[0], [ROWS, MPAD], [1, K]])
                    nc.vector.tensor_tensor(bx, ii, pgB, op=Alu.is_ge)
                    blc = gridp.tile([P, MPAD, K], F16, tag="blc")
                    nc.vector.tensor_tensor(blc, bx, codeT[:, :, :], op=Alu.mult)

                    # ---- slot-code max tree 4 -> 2 -> 1
                    t2 = gridp.tile([P, 2, K], F16, tag="t2")
                    nc.vector.tensor_tensor(t2, blc[:, 0:2, :], blc[:, 2:4, :], op=Alu.max)
                    nc.vector.tensor_tensor(
                        smax_i[:, c0:c0 + K], t2[:, 0, :], t2[:, 1, :], op=Alu.max
                    )

                    # ---- select S at winning slot: max_j [blc==smax]*(S+16)
                    sm = smax_i[:, :]
                    smB = _bc(sm, c0, [sm.ap[0], [0, MPAD], [1, K]])
                    eq = gridp.tile([P, MPAD, K], F16, tag="eq")
                    nc.vector.tensor_tensor(eq, blc, smB, op=Alu.is_equal)
                    slw = gridp.tile([P, MPAD, K], F16, tag="slw")
                    sga = sgt[:, :, :]
                    sgB = _bc(sga, c0, [sga.ap[0], [ROWS, MPAD], [1, K]])
                    nc.vector.tensor_tensor(slw, eq, sgB, op=Alu.mult)
                    s2 = gridp.tile([P, 2, K], F16, tag="s2")
                    nc.vector.tensor_tensor(s2, slw[:, 0:2, :], slw[:, 2:4, :], op=Alu.max)
                    nc.vector.tensor_tensor(
                        sl_i[:, c0:c0 + K], s2[:, 0, :], s2[:, 1, :], op=Alu.max
                    )

                    # ---- CE: exp + halving-tree sum over 80 classes,
                    # in half-chunks so DMA/Act/DVE pipeline finely
                    KH = K // 2
                    for hk in range(2):
                        h0 = c0 + hk * KH
                        sck = chunkp.tile([P, KH, C], F8, tag="sck")
                        nc.sync.dma_start(out=sck, in_=s_in[b, :, h0:h0 + KH, :])
                        esc = chunkp.tile([P, KH, C], F16, tag="esc")
                        nc.scalar.activation(esc, sck, Act.Exp)
                        e40 = chunkp.tile([P, KH, 40], F16, tag="e40")
                        nc.vector.tensor_tensor(e40, esc[:, :, 0:40], esc[:, :, 40:80], op=Alu.add)
                        e20 = chunkp.tile([P, KH, 20], F16, tag="e20")
                        nc.vector.tensor_tensor(e20, e40[:, :, 0:20], e40[:, :, 20:40], op=Alu.add)
                        e10 = chunkp.tile([P, KH, 10], F16, tag="e10")
                        nc.vector.tensor_tensor(e10, e20[:, :, 0:10], e20[:, :, 10:20], op=Alu.add)
                        e5 = chunkp.tile([P, KH, 5], F16, tag="e5")
                        nc.vector.tensor_tensor(e5, e10[:, :, 0:5], e10[:, :, 5:10], op=Alu.add)
                        nc.vector.reduce_sum(se_i[:, h0:h0 + KH], e5, axis=AX.X)

                # ship per-pred (sl+16, smax, se) rows when the image finishes;
                # host does ln + masked mean
                nc.sync.dma_start(out=o_sl[b], in_=sl_i)
                nc.sync.dma_start(out=o_sm[b], in_=smax_i)
                nc.sync.dma_start(out=o_se[b], in_=se_i)

    nc.compile()
    return nc


def _host_prep(preds, gtruths):
    """Spatial binning + fp16 feature building for all B images."""
    T = THR
    c_all = np.zeros((B, P, 4, ROWS), dtype=np.float16)
    s_all = np.zeros((B, P, ROWS, C), dtype=ml_dtypes.float8_e4m3)
    sg_all = np.zeros((B, P, MPAD, ROWS), dtype=np.float16)
    pg_all = np.zeros((B, P, MPAD, ROWS), dtype=np.float16)
    g_all = np.zeros((B, P, 4, MPAD), dtype=np.float16)
    for b in range(B):
        pb = preds[b, :, :4].astype(np.float64)
        sc = preds[b, :, 5:]
        g = gtruths[b, :, :4].astype(np.float64)
        gcls = gtruths[b, :, 4].astype(np.int64)
        pa = (pb[:, 2] - pb[:, 0]) * (pb[:, 3] - pb[:, 1])
        ga = (g[:, 2] - g[:, 0]) * (g[:, 3] - g[:, 1])
        cxc = (pb[:, 0] + pb[:, 2]) * 0.5
        ordx = np.argsort(cxc, kind="stable")
        cell_id = 0
        for i in range(CX):
            col = ordx[i * (N // CX):(i + 1) * (N // CX)]
            cyc = (pb[col, 1] + pb[col, 3]) * 0.5
            ordy = col[np.argsort(cyc, kind="stable")]
            for j in range(CY):
                cell = ordy[j * ROWS:(j + 1) * ROWS]
                x1, y1 = pb[cell, 0].min(), pb[cell, 1].min()
                x2, y2 = pb[cell, 2].max(), pb[cell, 3].max()
                wx = np.minimum(x2, g[:, 2]) - np.maximum(x1, g[:, 0])
                wy = np.minimum(y2, g[:, 3]) - np.maximum(y1, g[:, 1])
                ovl = np.clip(wx, 0, None) * np.clip(wy, 0, None)
                pamin = pa[cell].min()
                cand = (
                    (wx > 0) & (wy > 0)
                    & (ovl >= 0.97 * T * (pamin + ga))
                    & (ga * (1 - 0.97 * T) >= 0.97 * T * pamin)
                )
                idx = np.where(cand)[0]
                rank = ovl[idx] / (pamin + ga[idx])
                keep = idx[np.argsort(-rank)][:MPAD]
                nk = len(keep)
                c_all[b, cell_id, 0, :] = pb[cell, 2]
                c_all[b, cell_id, 1, :] = -pb[cell, 0]
                c_all[b, cell_id, 2, :] = pb[cell, 3]
                c_all[b, cell_id, 3, :] = -pb[cell, 1]
                s_all[b, cell_id, :, :] = sc[cell]
                gap_full = np.full(MPAD, DGA)
                gtab = g_all[b, cell_id]
                if nk:
                    gtab[0, :nk] = g[keep, 2]
                    gtab[1, :nk] = -g[keep, 0]
                    gtab[2, :nk] = g[keep, 3]
                    gtab[3, :nk] = -g[keep, 1]
                    gap_full[:nk] = ga[keep] / 3.5
                    sg_all[b, cell_id, :nk, :] = (sc[np.ix_(cell, gcls[keep])] + 16.0).T
                pg_all[b, cell_id, :, :] = gap_full[:, None] + (pa[cell] / 3.5)[None, :]
                cell_id += 1
    return c_all, s_all, sg_all, pg_all, g_all


def kernel(preds: np.ndarray, gtruths: np.ndarray) -> np.ndarray:
    if "nc" not in _CACHE:
        _CACHE["nc"] = _build()
    nc = _CACHE["nc"]

    preds = np.ascontiguousarray(preds, dtype=np.float32)
    gtruths = np.ascontiguousarray(gtruths, dtype=np.float32)
    c_all, s_all, sg_all, pg_all, g_all = _host_prep(preds, gtruths)

    in_maps = [
        {
            "c": c_all[c * IMGS_PER_CORE:(c + 1) * IMGS_PER_CORE],
            "s": s_all[c * IMGS_PER_CORE:(c + 1) * IMGS_PER_CORE],
            "sg": sg_all[c * IMGS_PER_CORE:(c + 1) * IMGS_PER_CORE],
            "pg": pg_all[c * IMGS_PER_CORE:(c + 1) * IMGS_PER_CORE],
            "g": g_all[c * IMGS_PER_CORE:(c + 1) * IMGS_PER_CORE],
        }
        for c in range(NCORES)
    ]
    res = run_bass_kernel_spmd(nc, in_maps, core_ids=list(range(NCORES)))
    _CACHE["last_result"] = res

    per_img = []
    for c in range(NCORES):
        r = res.results[c]
        for b in range(IMGS_PER_CORE):
            se = r["ose"][b].astype(np.float64)          # [P, ROWS]
            sl16 = r["osl"][b].astype(np.float64)        # sl + 16
            smax = r["osm"][b].astype(np.float64)
            valid = smax >= 0.5
            ce = (np.log(se) + 16.0) - sl16
            cnt = float(valid.sum())
            per_img.append(float((ce * valid).sum()) / max(cnt, 1.0))
    return np.asarray(np.mean(per_img), dtype=np.float32)
